# revision 24
# baseline (speedup 1.0000x reference)
"""CrossAttention Trainium2 kernel (8 NeuronCores, SPMD), bf16 compute.

Sharding: data-parallel over batch B=2, tensor-parallel over the 16 heads in
4 groups of 4 heads -> 8 cores, one (batch, head-group) pair each. Each core
computes its 4 heads' Q/K/V projections, masked softmax cross-attention, and
its partial output projection y_g = softmax(q k^T * scale) v @ Wo[:, g].T.
The host sums the 4 partial outputs per batch (the Wo row-split all-reduce,
done at unshard time) and adds the v-bias term Wo @ b_v, which is constant
across rows and factors out of the attention (softmax rows sum to 1).

Numerics: inputs are cast to bf16 on the host; every matmul runs bf16 x bf16
with fp32 PSUM accumulation; softmax statistics stay fp32 except the
broadcast reciprocal (bf16).

Layout: the PE contracts over the partition dim, so activations and weights
arrive contraction-major (pre-transposed on the host); every device DMA is a
plain strided row load, no device transposes. Attention is computed
scores-transposed: ST[m, n] per head, so the PV matmul contracts over m
directly and the denominator comes free from an appended ones-column on the
v stationary operand. exp() is unnormalized; mask zeros are applied
multiplicatively after exp on the DVE with step-1 APs (2x packed rate).

Schedule (v2): one continuous software pipeline in (pair, n-chunk) units so
the ACT exp stream (the serial bottleneck, ~73us) never idles and the PE
never gaps long enough for the HAM clock gate to re-throttle:
  warmup MMs (HAM) | Qproj | Kproj(et0)
  S(0,c0) + Kproj(et1) + Vproj      # S = scores->exp->mask sweep over 16 mt
  S(0,c1) + PV(0,c0)
  S(1,c0) + PV(0,c1) + norm(p0,c0)
  S(1,c1) + PV(1,c0) + norm(p0,c1)
  PV(1,c1) + norm(p1,c0)
  outproj(c0) | norm(p1,c1) | outproj(c1) | streamed y stores
Scores matmuls run row-tiled (two 64-contraction heads concurrently in the
top/bottom array halves). Softmax normalization broadcasts the bf16
reciprocal across partitions with a rank-1 ones matmul instead of gpsimd
partition_broadcast. Normalize work is emitted a few iterations into the
following unit so its instructions never head-block an engine FIFO before
their dependencies resolve. All input DMAs are single batched descriptors on
the sync/vector/gpsimd queues; the scalar queue carries only ACTIVATEs until
the tail, where it helps evict PSUM.
"""

import os

import numpy as np
import ml_dtypes

import concourse.bass as bass
import concourse.bacc as bacc
import concourse.mybir as mybir
import concourse.tile as tile
from concourse.bass_utils import run_bass_kernel_spmd

DIM = 1024
HEAD_DIM = 64
NUM_HEADS = 16
SCALE = HEAD_DIM**-0.5
B, N, M = 2, 1024, 2048
HPC = 4  # heads per core
E = HPC * HEAD_DIM  # 256: per-core projection width
P = 128
F32 = mybir.dt.float32
BF16 = mybir.dt.bfloat16
FP8 = mybir.dt.float8e4
CT = DIM // P  # 8 contraction tiles
MT = M // P  # 16 m tiles
NC = N // 512  # 2 n-chunks


def build_program():
    nc = bacc.Bacc("TRN2", target_bir_lowering=False, debug=False, num_devices=8)

    # all activation/weight shards arrive contraction-major (pre-transposed)
    xT_d = nc.dram_tensor("xT", [DIM, N], FP8, kind="ExternalInput").ap()
    ctxr_d = nc.dram_tensor(
        "ctxr", [8, P, CT, 256], BF16, kind="ExternalInput"
    ).ap()
    maskr_d = nc.dram_tensor(
        "maskr", [4, P, 4, N], BF16, kind="ExternalInput"
    ).ap()
    wqT_d = nc.dram_tensor("wqT", [P, CT, E], FP8, kind="ExternalInput").ap()
    wkT_d = nc.dram_tensor("wkT", [P, CT, E], BF16, kind="ExternalInput").ap()
    wvT_d = nc.dram_tensor("wvT", [P, CT, E], BF16, kind="ExternalInput").ap()
    woT_d = nc.dram_tensor("woT", [P, E // P, DIM], BF16, kind="ExternalInput").ap()
    bk_d = nc.dram_tensor("bk", [E], F32, kind="ExternalInput").ap()
    y_d = nc.dram_tensor("y", [N, DIM], BF16, kind="ExternalOutput").ap()

    Exp = mybir.ActivationFunctionType.Exp

    from contextlib import ExitStack

    with tile.TileContext(nc) as tc, ExitStack() as ctx:
        const = ctx.enter_context(tc.tile_pool(name="const", bufs=1))
        bk_sb = const.tile([P, E // P], F32)
        ones_t = const.tile([1, HEAD_DIM], BF16)
        warm = const.tile([P, P], BF16)
        nc.vector.memset(ones_t, 1.0)
        nc.vector.memset(warm, 0.0)

        persist = ctx.enter_context(tc.tile_pool(name="persist", bufs=1))
        qT = persist.tile([P, E // P, N], BF16)
        kT = persist.tile([P, E // P, M], BF16)
        vaug = persist.tile([P, MT, HPC, HEAD_DIM + 1], BF16)
        masks = persist.tile([P, MT, N], BF16)
        exmst = persist.tile([P, MT, 2, N], BF16)
        # rows 0:64 unnormalized attention out, row 64 denominator
        ot_sb = persist.tile([HEAD_DIM + 1, HPC, N], F32)
        otn2 = persist.tile([P, E // P, N], BF16)

        # ones column: fill everything; v evictions overwrite cols 0:64
        nc.vector.memset(vaug, 1.0)

        # ---------- HAM warmup: keep the PE busy while input DMAs land ----
        with tc.tile_pool(name="wps", bufs=1, space="PSUM") as wpsp:
            wps = wpsp.tile([P, P], F32)
            for _ in range(180):
                nc.tensor.matmul(wps, lhsT=warm, rhs=warm, start=True, stop=True)

        # ---------- batched input DMAs (one descriptor per tensor) --------
        # the scalar queue issues the ctx-side loads at t=0 (cheap descriptor
        # generation) and then carries only the exp ACTIVATE stream.
        nc.gpsimd.dma_start(out=bk_sb, in_=bk_d.rearrange("(t p) -> p t", p=P))

        exp_pool = ctx.enter_context(tc.tile_pool(name="exp", bufs=2))

        def emit_scores(spool, mt, hp, chn):
            """row-tiled scores pair -> exp -> mask for (pair hp, chunk chn)."""
            st = spool.tile([P, 2, 512], F32, tag="st", name="st", bufs=2)
            for hl in range(2):
                erow = slice(hl * HEAD_DIM, (hl + 1) * HEAD_DIM)
                nc.tensor.matmul(
                    st[:, hl, :],
                    lhsT=kT[erow, hp, mt * P : (mt + 1) * P],
                    rhs=qT[erow, hp, chn * 512 : (chn + 1) * 512],
                    start=True,
                    stop=True,
                )
            ex = exp_pool.tile([P, 2, 512], BF16, tag="ex", name="ex")
            nc.scalar.activation(ex, st, Exp, scale=float(SCALE))
            for hl in range(2):
                nc.vector.tensor_mul(
                    exmst[:, mt, hl, chn * 512 : (chn + 1) * 512],
                    ex[:, hl, :],
                    masks[:, mt, chn * 512 : (chn + 1) * 512],
                )

        def emit_pv(ot_ps, hp, chn, mt):
            for hl in range(2):
                h = hp * 2 + hl
                nc.tensor.matmul(
                    ot_ps[hl],
                    lhsT=vaug[:, mt, h, :],
                    rhs=exmst[:, mt, hl, chn * 512 : (chn + 1) * 512],
                    start=(mt == 0),
                    stop=(mt == MT - 1),
                )

        def evict_ot(ot_ps, hp, chn, engs):
            for hl in range(2):
                dst = ot_sb[:, hp * 2 + hl, chn * 512 : (chn + 1) * 512]
                eng = engs[hl % len(engs)]
                if eng is nc.scalar:
                    eng.copy(dst, ot_ps[hl])
                else:
                    eng.tensor_copy(dst, ot_ps[hl])

        def normalize(rbps_pool, hp, chn, ring=None):
            """softmax-normalize heads (2hp, 2hp+1) on n-chunk chn."""
            ring = ring or nc.gpsimd
            cs = slice(chn * 512, (chn + 1) * 512)
            dn = dnp.tile([2, 512], F32, tag="dn", name="dn")
            for hl in range(2):
                ring.dma_start(
                    out=dn[hl : hl + 1, :],
                    in_=ot_sb[HEAD_DIM : HEAD_DIM + 1, hp * 2 + hl, cs],
                )
            rc = dnp.tile([2, 512], F32, tag="rc", name="rc")
            nc.vector.reciprocal_approx_fast(out=rc, in_=dn)
            rcb = dnp.tile([2, 512], BF16, tag="rcb", name="rcb")
            nc.vector.tensor_copy(rcb, rc)
            rc1 = dnp.tile([1, 2, 512], BF16, tag="rc1", name="rc1")
            ring.dma_start(out=rc1, in_=rcb)
            for hl in range(2):
                rbps = rbps_pool.tile([HEAD_DIM, 512], F32, tag="rb")
                nc.tensor.matmul(
                    rbps,
                    lhsT=ones_t,
                    rhs=rc1[0:1, hl, :],
                    start=True,
                    stop=True,
                )
                if hl == 0:
                    nc.vector.tensor_mul(
                        otn2[:HEAD_DIM, hp, cs],
                        ot_sb[:HEAD_DIM, hp * 2, cs],
                        rbps,
                    )
                else:
                    tmp = rbp.tile([HEAD_DIM, 512], BF16, tag="tmp", name="tmp")
                    nc.vector.tensor_mul(
                        tmp, ot_sb[:HEAD_DIM, hp * 2 + 1, cs], rbps
                    )
                    # partition shift 0:64 -> 64:128 via SBUF-SBUF DMA
                    ring.dma_start(out=otn2[HEAD_DIM:P, hp, cs], in_=tmp)

        with tc.tile_pool(name="spool", bufs=1, space="PSUM") as spool:
            with tc.tile_pool(name="wctx", bufs=1) as wctx_pool:
                wkT = wctx_pool.tile([P, CT, E], BF16)
                wvT = wctx_pool.tile([P, CT, E], BF16)
                ctxT = wctx_pool.tile([P, 8, CT, 256], BF16)

                def emit_kproj(kps, et, chm):
                    pk = kps.tile([P, 512], F32, tag="pk", name="pk")
                    for j in range(CT):
                        nc.tensor.matmul(
                            pk,
                            lhsT=wkT[:, j, et * P : (et + 1) * P],
                            rhs=ctxT[:, 2 * chm : 2 * chm + 2, j, :],
                            start=(j == 0),
                            stop=(j == CT - 1),
                        )
                    nc.vector.tensor_scalar_add(
                        kT[:, et, chm * 512 : (chm + 1) * 512],
                        pk,
                        bk_sb[:, et : et + 1],
                    )

                with tc.tile_pool(name="qx", bufs=1) as qx_pool:
                    wqT = qx_pool.tile([P, CT, E], FP8)
                    xT = qx_pool.tile([P, CT, N], FP8)

                    # all loads are host-pretiled so every DMA has a
                    # contiguous per-partition source (cheap descriptor gen,
                    # fat bursts); ctx loads m-chunk-major so Kproj(chm) and
                    # Vproj(mt) start as soon as their m-range lands. The
                    # scalar ring fronts ctx chunk 0 so Kproj(0,0) -> first
                    # scores -> the exp stream starts as early as possible.
                    # ring assignment (measured): sync ring carries wq +
                    # x-lo + masks; scalar ring carries x-hi + wk + wv + ctx
                    # m-chunks. The rings fair-share among queued transfers,
                    # so first-needed tensors are queued first.
                    nc.sync.dma_start(out=wqT, in_=wqT_d)
                    for j2 in range(0, CT, 2):
                        ring = nc.sync if j2 < 4 else nc.scalar
                        ring.dma_start(
                            out=xT[:, j2 : j2 + 2, :],
                            in_=xT_d[j2 * P : (j2 + 2) * P, :].rearrange(
                                "(j p) n -> p j n", p=P
                            ),
                        )
                    nc.scalar.dma_start(out=wkT, in_=wkT_d)
                    nc.scalar.dma_start(out=wvT, in_=wvT_d)
                    for mc in range(8):
                        nc.scalar.dma_start(out=ctxT[:, mc, :, :], in_=ctxr_d[mc])
                    for q in range(4):
                        nc.sync.dma_start(
                            out=masks[:, 4 * q : 4 * (q + 1), :], in_=maskr_d[q]
                        )

                    # ---------- Q projection ----------
                    with tc.tile_pool(name="ppsA", bufs=3, space="PSUM") as ppsA:
                        for et in range(E // P):
                            for chn in range(NC):
                                pq = ppsA.tile([P, 512], F32, tag="pq")
                                for j in range(CT):
                                    nc.tensor.matmul(
                                        pq,
                                        lhsT=wqT[:, j, et * P : (et + 1) * P],
                                        rhs=xT[:, j, chn * 512 : (chn + 1) * 512],
                                        start=(j == 0),
                                        stop=(j == CT - 1),
                                    )
                                nc.vector.tensor_copy(
                                    qT[:, et, chn * 512 : (chn + 1) * 512], pq
                                )

                # qx pool (xT, wqT) closed.
                with (
                    tc.tile_pool(name="kps", bufs=2, space="PSUM") as kps,
                    tc.tile_pool(name="vps", bufs=2, space="PSUM") as vps,
                ):
                    # S(0,c0) + Kproj just-in-time per m-chunk + Vproj
                    for mt in range(MT):
                        if mt % 4 == 0:
                            emit_kproj(kps, 0, mt // 4)
                        if mt % 4 == 2:
                            emit_kproj(kps, 1, mt // 4)
                        emit_scores(spool, mt, 0, 0)
                        pv = vps.tile([P, E], F32, tag="pv")
                        for j in range(CT):
                            nc.tensor.matmul(
                                pv,
                                lhsT=ctxT[
                                    :, mt // 2, j,
                                    (mt % 2) * P : (mt % 2 + 1) * P,
                                ],
                                rhs=wvT[:, j, :],
                                start=(j == 0),
                                stop=(j == CT - 1),
                            )
                        nc.vector.tensor_copy(vaug[:, mt, :, :HEAD_DIM], pv)
            # wctx closed: ctx-side SBUF freed for the tail pools.
            rbp = ctx.enter_context(tc.tile_pool(name="rbp", bufs=1))
            dnp = ctx.enter_context(tc.tile_pool(name="dnp", bufs=1))

            with (
                tc.tile_pool(name="rbpsA", bufs=2, space="PSUM") as rbpsA,
                tc.tile_pool(name="pvps", bufs=1, space="PSUM") as pvps,
            ):
                # units 2-4: scores(sp,sc) overlapped with the previous
                # pair-chunk's PV front-loaded 2-per-iteration, evicted and
                # normalized while the exp stream still runs.
                for (sp, sc), (vp, vc) in [
                    ((0, 1), (0, 0)),
                    ((1, 0), (0, 1)),
                    ((1, 1), (1, 0)),
                ]:
                    ot_ps = [
                        pvps.tile(
                            [HEAD_DIM + 1, 512], F32, tag=f"o{i}", name=f"o{i}"
                        )
                        for i in range(2)
                    ]
                    for mt in range(MT):
                        emit_scores(spool, mt, sp, sc)
                        if mt < MT // 2:
                            emit_pv(ot_ps, vp, vc, 2 * mt)
                            emit_pv(ot_ps, vp, vc, 2 * mt + 1)
                        if mt == 8:
                            evict_ot(ot_ps, vp, vc, [nc.vector])
                        if mt == 11:
                            normalize(rbpsA, vp, vc)

        # ---------- tail: PV(1,c1), normalize, output projection ----------
        with (
            tc.tile_pool(name="tailp", bufs=1) as tailp,
            tc.tile_pool(name="rbpsB", bufs=2, space="PSUM") as rbpsB,
            tc.tile_pool(name="pvpsB", bufs=1, space="PSUM") as pvpsB,
            tc.tile_pool(name="ypsum", bufs=3, space="PSUM") as ypsum,
            tc.tile_pool(name="ypool", bufs=3) as ypool,
        ):
            woT = tailp.tile([P, E // P, DIM], BF16)
            nc.gpsimd.dma_start(out=woT, in_=woT_d)
            ot_ps = [
                pvpsB.tile([HEAD_DIM + 1, 512], F32, tag=f"t{i}", name=f"t{i}")
                for i in range(2)
            ]

            def outproj(chn, i0):
                i = i0
                for nbl in range(4):
                    nb = chn * 4 + nbl
                    for oc in range(DIM // 512):
                        yp = ypsum.tile([P, 512], F32, tag="yp")
                        for hp in range(E // P):
                            nc.tensor.matmul(
                                yp,
                                lhsT=otn2[:, hp, nb * P : (nb + 1) * P],
                                rhs=woT[:, hp, oc * 512 : (oc + 1) * 512],
                                start=(hp == 0),
                                stop=(hp == E // P - 1),
                            )
                        ys = ypool.tile([P, 512], BF16, tag="ys")
                        if i % 2:
                            nc.scalar.copy(ys, yp)
                        else:
                            nc.vector.tensor_copy(ys, yp)
                        ring = nc.sync if i % 2 else nc.scalar
                        ring.dma_start(
                            out=y_d[
                                nb * P : (nb + 1) * P, oc * 512 : (oc + 1) * 512
                            ],
                            in_=ys,
                        )
                        i += 1

            outproj(0, 0)
            for mt in range(MT):
                emit_pv(ot_ps, 1, 1, mt)
            evict_ot(ot_ps, 1, 1, [nc.scalar, nc.vector])
            normalize(rbpsB, 1, 1)
            outproj(1, 8)

    nc.compile()
    return nc


_NC_CACHE = []


def _get_nc():
    if not _NC_CACHE:
        _NC_CACHE.append(build_program())
    return _NC_CACHE[0]


def make_in_maps(x, context, mask, Wq, Wkv, b_kv, Wo):
    bf = ml_dtypes.bfloat16
    x = np.asarray(x, dtype=np.float32)
    context = np.asarray(context, dtype=np.float32)
    mask = np.asarray(mask)
    Wq = np.asarray(Wq, dtype=np.float32)
    Wkv = np.asarray(Wkv, dtype=np.float32)
    b_kv = np.asarray(b_kv, dtype=np.float32)
    Wo = np.asarray(Wo, dtype=np.float32)

    def tile_w(w, dt=None):
        # [DIM, E] contraction-major -> [128, CT, E] partition-tiled
        return np.ascontiguousarray(
            w.reshape(-1, 128, w.shape[1]).transpose(1, 0, 2)
        ).astype(dt or bf)

    in_maps = []
    for b in range(B):
        xtb = np.ascontiguousarray(x[b].T).astype(ml_dtypes.float8_e4m3)
        ctb = context[b].T  # [DIM, M]
        # [4, 128, CT, 512]: m-chunk-major, partition-tiled, contiguous
        ctr = np.ascontiguousarray(
            ctb.reshape(8, 128, 8, 256).transpose(2, 1, 0, 3)
        ).astype(bf)
        # [4, 128, 4, N]: mask m-tiles grouped in chunks of 4
        mtb = mask[b].T  # [M, N]
        mtr = np.ascontiguousarray(
            mtb.reshape(4, 4, 128, N).transpose(0, 2, 1, 3)
        ).astype(bf)
        for g in range(NUM_HEADS // HPC):
            sl = slice(E * g, E * (g + 1))
            wog = Wo[:, sl].T  # [E, DIM]
            wor = np.ascontiguousarray(
                wog.reshape(2, 128, DIM).transpose(1, 0, 2)
            ).astype(bf)
            in_maps.append(
                {
                    "xT": xtb,
                    "ctxr": ctr,
                    "maskr": mtr,
                    "wqT": tile_w(Wq[sl].T, ml_dtypes.float8_e4m3),
                    "wkT": tile_w(Wkv[sl].T),
                    "wvT": tile_w(Wkv[DIM + E * g : DIM + E * (g + 1)].T),
                    "woT": wor,
                    "bk": np.ascontiguousarray(b_kv[sl]),
                }
            )
    return in_maps


def combine_outputs(ys, b_kv, Wo):
    """ys: list of 8 per-core partial outputs [N, DIM], core order (b, g)."""
    b_v = np.asarray(b_kv, dtype=np.float32)[DIM:]
    ybias = np.asarray(Wo, dtype=np.float32) @ b_v  # [DIM]
    out = np.empty((B, N, DIM), dtype=np.float32)
    G = NUM_HEADS // HPC
    for b in range(B):
        acc = np.asarray(ys[G * b], dtype=np.float32).copy()  # bf16 -> f32
        for g in range(1, G):
            acc += ys[G * b + g]
        out[b] = acc + ybias[None, :]
    return out


def kernel(x, context, mask, Wq, Wkv, b_kv, Wo):
    nc = _get_nc()
    in_maps = make_in_maps(x, context, mask, Wq, Wkv, b_kv, Wo)
    res = run_bass_kernel_spmd(nc, in_maps, core_ids=list(range(8)))
    ys = [m["y"] for m in res.results]
    return combine_outputs(ys, b_kv, Wo)


# revision 25
# speedup vs baseline: 1.1748x; 1.1748x over previous
"""CrossAttention Trainium2 kernel (8 NeuronCores, SPMD), bf16 compute.

Sharding: data-parallel over batch B=2, tensor-parallel over the 16 heads in
4 groups of 4 heads -> 8 cores, one (batch, head-group) pair each. Each core
computes its 4 heads' Q/K/V projections, masked softmax cross-attention, and
its partial output projection y_g = softmax(q k^T * scale) v @ Wo[:, g].T.
The host sums the 4 partial outputs per batch (the Wo row-split all-reduce,
done at unshard time) and adds the v-bias term Wo @ b_v, which is constant
across rows and factors out of the attention (softmax rows sum to 1).

Numerics: inputs are cast to bf16 on the host; every matmul runs bf16 x bf16
with fp32 PSUM accumulation; softmax statistics stay fp32 except the
broadcast reciprocal (bf16).

Layout: the PE contracts over the partition dim, so activations and weights
arrive contraction-major (pre-transposed on the host); every device DMA is a
plain strided row load, no device transposes. Attention is computed
scores-transposed: ST[m, n] per head, so the PV matmul contracts over m
directly and the denominator comes free from an appended ones-column on the
v stationary operand. exp() is unnormalized; mask zeros are applied
multiplicatively after exp on the DVE with step-1 APs (2x packed rate).

Schedule (v2): one continuous software pipeline in (pair, n-chunk) units so
the ACT exp stream (the serial bottleneck, ~73us) never idles and the PE
never gaps long enough for the HAM clock gate to re-throttle:
  warmup MMs (HAM) | Qproj | Kproj(et0)
  S(0,c0) + Kproj(et1) + Vproj      # S = scores->exp->mask sweep over 16 mt
  S(0,c1) + PV(0,c0)
  S(1,c0) + PV(0,c1) + norm(p0,c0)
  S(1,c1) + PV(1,c0) + norm(p0,c1)
  PV(1,c1) + norm(p1,c0)
  outproj(c0) | norm(p1,c1) | outproj(c1) | streamed y stores
Scores matmuls run row-tiled (two 64-contraction heads concurrently in the
top/bottom array halves). Softmax normalization broadcasts the bf16
reciprocal across partitions with a rank-1 ones matmul instead of gpsimd
partition_broadcast. Normalize work is emitted a few iterations into the
following unit so its instructions never head-block an engine FIFO before
their dependencies resolve. All input DMAs are single batched descriptors on
the sync/vector/gpsimd queues; the scalar queue carries only ACTIVATEs until
the tail, where it helps evict PSUM.
"""

import os

import numpy as np
import ml_dtypes

import concourse.bass as bass
import concourse.bacc as bacc
import concourse.mybir as mybir
import concourse.tile as tile
from concourse.bass_utils import run_bass_kernel_spmd

DIM = 1024
HEAD_DIM = 64
NUM_HEADS = 16
SCALE = HEAD_DIM**-0.5
B, N, M = 2, 1024, 2048
HPC = 4  # heads per core
E = HPC * HEAD_DIM  # 256: per-core projection width
P = 128
F32 = mybir.dt.float32
BF16 = mybir.dt.bfloat16
FP8 = mybir.dt.float8e4
CT = DIM // P  # 8 contraction tiles
MT = M // P  # 16 m tiles
NC = N // 512  # 2 n-chunks


def build_program():
    nc = bacc.Bacc("TRN2", target_bir_lowering=False, debug=False, num_devices=8)

    # all activation/weight shards arrive contraction-major (pre-transposed)
    xT_d = nc.dram_tensor("xT", [DIM, N], BF16, kind="ExternalInput").ap()
    ctxr_d = nc.dram_tensor(
        "ctxr", [4, P, CT, 512], BF16, kind="ExternalInput"
    ).ap()
    maskr_d = nc.dram_tensor(
        "maskr", [4, P, 4, N], BF16, kind="ExternalInput"
    ).ap()
    wqT_d = nc.dram_tensor("wqT", [P, CT, E], BF16, kind="ExternalInput").ap()
    wkT_d = nc.dram_tensor("wkT", [P, CT, E], BF16, kind="ExternalInput").ap()
    wvT_d = nc.dram_tensor("wvT", [P, CT, E], BF16, kind="ExternalInput").ap()
    woT_d = nc.dram_tensor("woT", [P, E // P, DIM], BF16, kind="ExternalInput").ap()
    bk_d = nc.dram_tensor("bk", [E], F32, kind="ExternalInput").ap()
    y_d = nc.dram_tensor("y", [N, DIM], BF16, kind="ExternalOutput").ap()

    Exp = mybir.ActivationFunctionType.Exp

    from contextlib import ExitStack

    with tile.TileContext(nc) as tc, ExitStack() as ctx:
        const = ctx.enter_context(tc.tile_pool(name="const", bufs=1))
        bk_sb = const.tile([P, E // P], F32)
        ones_t = const.tile([1, HEAD_DIM], BF16)
        warm = const.tile([P, P], BF16)
        nc.vector.memset(ones_t, 1.0)
        nc.vector.memset(warm, 0.0)

        persist = ctx.enter_context(tc.tile_pool(name="persist", bufs=1))
        qT = persist.tile([P, E // P, N], BF16)
        kT = persist.tile([P, E // P, M], BF16)
        vaug = persist.tile([P, MT, HPC, HEAD_DIM + 1], BF16)
        masks = persist.tile([P, MT, N], BF16)
        exmst = persist.tile([P, MT, 2, N], BF16)
        # rows 0:64 unnormalized attention out, row 64 denominator
        ot_sb = persist.tile([HEAD_DIM + 1, HPC, N], F32)
        otn2 = persist.tile([P, E // P, N], BF16)

        # ones column: fill everything; v evictions overwrite cols 0:64
        nc.vector.memset(vaug, 1.0)

        # ---------- HAM warmup: keep the PE busy while input DMAs land ----
        with tc.tile_pool(name="wps", bufs=1, space="PSUM") as wpsp:
            wps = wpsp.tile([P, P], F32)
            for _ in range(180):
                nc.tensor.matmul(wps, lhsT=warm, rhs=warm, start=True, stop=True)

        # ---------- batched input DMAs (one descriptor per tensor) --------
        # the scalar queue issues the ctx-side loads at t=0 (cheap descriptor
        # generation) and then carries only the exp ACTIVATE stream.
        nc.gpsimd.dma_start(out=bk_sb, in_=bk_d.rearrange("(t p) -> p t", p=P))

        exp_pool = ctx.enter_context(tc.tile_pool(name="exp", bufs=2))

        def emit_scores(spool, mt, hp, chn):
            """row-tiled scores pair -> exp -> mask for (pair hp, chunk chn)."""
            st = spool.tile([P, 2, 512], F32, tag="st", name="st", bufs=2)
            for hl in range(2):
                erow = slice(hl * HEAD_DIM, (hl + 1) * HEAD_DIM)
                nc.tensor.matmul(
                    st[:, hl, :],
                    lhsT=kT[erow, hp, mt * P : (mt + 1) * P],
                    rhs=qT[erow, hp, chn * 512 : (chn + 1) * 512],
                    start=True,
                    stop=True,
                )
            ex = exp_pool.tile([P, 2, 512], BF16, tag="ex", name="ex")
            nc.scalar.activation(ex, st, Exp, scale=float(SCALE))
            for hl in range(2):
                nc.vector.tensor_mul(
                    exmst[:, mt, hl, chn * 512 : (chn + 1) * 512],
                    ex[:, hl, :],
                    masks[:, mt, chn * 512 : (chn + 1) * 512],
                )

        def emit_pv(ot_ps, hp, chn, mt):
            for hl in range(2):
                h = hp * 2 + hl
                nc.tensor.matmul(
                    ot_ps[hl],
                    lhsT=vaug[:, mt, h, :],
                    rhs=exmst[:, mt, hl, chn * 512 : (chn + 1) * 512],
                    start=(mt == 0),
                    stop=(mt == MT - 1),
                )

        def evict_ot(ot_ps, hp, chn, engs):
            for hl in range(2):
                dst = ot_sb[:, hp * 2 + hl, chn * 512 : (chn + 1) * 512]
                eng = engs[hl % len(engs)]
                if eng is nc.scalar:
                    eng.copy(dst, ot_ps[hl])
                else:
                    eng.tensor_copy(dst, ot_ps[hl])

        def normalize(rbps_pool, hp, chn, ring=None):
            """softmax-normalize heads (2hp, 2hp+1) on n-chunk chn."""
            ring = ring or nc.gpsimd
            cs = slice(chn * 512, (chn + 1) * 512)
            dn = dnp.tile([2, 512], F32, tag="dn", name="dn")
            for hl in range(2):
                ring.dma_start(
                    out=dn[hl : hl + 1, :],
                    in_=ot_sb[HEAD_DIM : HEAD_DIM + 1, hp * 2 + hl, cs],
                )
            rc = dnp.tile([2, 512], F32, tag="rc", name="rc")
            nc.vector.reciprocal_approx_fast(out=rc, in_=dn)
            rcb = dnp.tile([2, 512], BF16, tag="rcb", name="rcb")
            nc.vector.tensor_copy(rcb, rc)
            rc1 = dnp.tile([1, 2, 512], BF16, tag="rc1", name="rc1")
            ring.dma_start(out=rc1, in_=rcb)
            for hl in range(2):
                rbps = rbps_pool.tile([HEAD_DIM, 512], F32, tag="rb")
                nc.tensor.matmul(
                    rbps,
                    lhsT=ones_t,
                    rhs=rc1[0:1, hl, :],
                    start=True,
                    stop=True,
                )
                if hl == 0:
                    nc.vector.tensor_mul(
                        otn2[:HEAD_DIM, hp, cs],
                        ot_sb[:HEAD_DIM, hp * 2, cs],
                        rbps,
                    )
                else:
                    tmp = rbp.tile([HEAD_DIM, 512], BF16, tag="tmp", name="tmp")
                    nc.vector.tensor_mul(
                        tmp, ot_sb[:HEAD_DIM, hp * 2 + 1, cs], rbps
                    )
                    # partition shift 0:64 -> 64:128 via SBUF-SBUF DMA
                    ring.dma_start(out=otn2[HEAD_DIM:P, hp, cs], in_=tmp)

        with tc.tile_pool(name="spool", bufs=1, space="PSUM") as spool:
            with tc.tile_pool(name="wctx", bufs=1) as wctx_pool:
                wkT = wctx_pool.tile([P, CT, E], BF16)
                wvT = wctx_pool.tile([P, CT, E], BF16)
                ctxT = wctx_pool.tile([P, 4, CT, 512], BF16)

                def emit_kproj(kps, et, chm):
                    pk = kps.tile([P, 512], F32, tag="pk", name="pk")
                    for j in range(CT):
                        nc.tensor.matmul(
                            pk,
                            lhsT=wkT[:, j, et * P : (et + 1) * P],
                            rhs=ctxT[:, chm, j, :],
                            start=(j == 0),
                            stop=(j == CT - 1),
                        )
                    nc.vector.tensor_scalar_add(
                        kT[:, et, chm * 512 : (chm + 1) * 512],
                        pk,
                        bk_sb[:, et : et + 1],
                    )

                with tc.tile_pool(name="qx", bufs=1) as qx_pool:
                    wqT = qx_pool.tile([P, CT, E], BF16)
                    xT = qx_pool.tile([P, CT, N], BF16)

                    # all loads are host-pretiled so every DMA has a
                    # contiguous per-partition source (cheap descriptor gen,
                    # fat bursts); ctx loads m-chunk-major so Kproj(chm) and
                    # Vproj(mt) start as soon as their m-range lands. The
                    # scalar ring fronts ctx chunk 0 so Kproj(0,0) -> first
                    # scores -> the exp stream starts as early as possible.
                    # ring assignment (measured): sync ring carries wq +
                    # x-lo + masks; scalar ring carries x-hi + wk + wv + ctx
                    # m-chunks. The rings fair-share among queued transfers,
                    # so first-needed tensors are queued first.
                    nc.sync.dma_start(out=wqT, in_=wqT_d)
                    for j2 in range(0, CT, 2):
                        ring = nc.sync if j2 < 4 else nc.scalar
                        ring.dma_start(
                            out=xT[:, j2 : j2 + 2, :],
                            in_=xT_d[j2 * P : (j2 + 2) * P, :].rearrange(
                                "(j p) n -> p j n", p=P
                            ),
                        )
                    nc.scalar.dma_start(out=wkT, in_=wkT_d)
                    nc.scalar.dma_start(out=wvT, in_=wvT_d)
                    for mc in range(4):
                        nc.scalar.dma_start(out=ctxT[:, mc, :, :], in_=ctxr_d[mc])
                    for q in range(4):
                        nc.sync.dma_start(
                            out=masks[:, 4 * q : 4 * (q + 1), :], in_=maskr_d[q]
                        )

                    # ---------- Q projection ----------
                    with tc.tile_pool(name="ppsA", bufs=3, space="PSUM") as ppsA:
                        for et in range(E // P):
                            for chn in range(NC):
                                pq = ppsA.tile([P, 512], F32, tag="pq")
                                for j in range(CT):
                                    nc.tensor.matmul(
                                        pq,
                                        lhsT=wqT[:, j, et * P : (et + 1) * P],
                                        rhs=xT[:, j, chn * 512 : (chn + 1) * 512],
                                        start=(j == 0),
                                        stop=(j == CT - 1),
                                    )
                                nc.vector.tensor_copy(
                                    qT[:, et, chn * 512 : (chn + 1) * 512], pq
                                )

                # qx pool (xT, wqT) closed.
                with (
                    tc.tile_pool(name="kps", bufs=2, space="PSUM") as kps,
                    tc.tile_pool(name="vps", bufs=2, space="PSUM") as vps,
                ):
                    # S(0,c0) + Kproj just-in-time per m-chunk + Vproj
                    for mt in range(MT):
                        if mt % 4 == 0:
                            emit_kproj(kps, 0, mt // 4)
                        if mt % 4 == 2:
                            emit_kproj(kps, 1, mt // 4)
                        emit_scores(spool, mt, 0, 0)
                        pv = vps.tile([P, E], F32, tag="pv")
                        for j in range(CT):
                            nc.tensor.matmul(
                                pv,
                                lhsT=ctxT[
                                    :, mt // 4, j,
                                    (mt % 4) * P : (mt % 4 + 1) * P,
                                ],
                                rhs=wvT[:, j, :],
                                start=(j == 0),
                                stop=(j == CT - 1),
                            )
                        nc.vector.tensor_copy(vaug[:, mt, :, :HEAD_DIM], pv)
            # wctx closed: ctx-side SBUF freed for the tail pools.
            rbp = ctx.enter_context(tc.tile_pool(name="rbp", bufs=1))
            dnp = ctx.enter_context(tc.tile_pool(name="dnp", bufs=1))

            with (
                tc.tile_pool(name="rbpsA", bufs=2, space="PSUM") as rbpsA,
                tc.tile_pool(name="pvps", bufs=1, space="PSUM") as pvps,
            ):
                # units 2-4: scores(sp,sc) overlapped with the previous
                # pair-chunk's PV front-loaded 2-per-iteration, evicted and
                # normalized while the exp stream still runs.
                for (sp, sc), (vp, vc) in [
                    ((0, 1), (0, 0)),
                    ((1, 0), (0, 1)),
                    ((1, 1), (1, 0)),
                ]:
                    ot_ps = [
                        pvps.tile(
                            [HEAD_DIM + 1, 512], F32, tag=f"o{i}", name=f"o{i}"
                        )
                        for i in range(2)
                    ]
                    for mt in range(MT):
                        emit_scores(spool, mt, sp, sc)
                        if mt < MT // 2:
                            emit_pv(ot_ps, vp, vc, 2 * mt)
                            emit_pv(ot_ps, vp, vc, 2 * mt + 1)
                        if mt == 8:
                            evict_ot(ot_ps, vp, vc, [nc.vector])
                        if mt == 11:
                            normalize(rbpsA, vp, vc)

        # ---------- tail: PV(1,c1), normalize, output projection ----------
        with (
            tc.tile_pool(name="tailp", bufs=1) as tailp,
            tc.tile_pool(name="rbpsB", bufs=2, space="PSUM") as rbpsB,
            tc.tile_pool(name="pvpsB", bufs=1, space="PSUM") as pvpsB,
            tc.tile_pool(name="ypsum", bufs=3, space="PSUM") as ypsum,
            tc.tile_pool(name="ypool", bufs=3) as ypool,
        ):
            woT = tailp.tile([P, E // P, DIM], BF16)
            nc.gpsimd.dma_start(out=woT, in_=woT_d)
            ot_ps = [
                pvpsB.tile([HEAD_DIM + 1, 512], F32, tag=f"t{i}", name=f"t{i}")
                for i in range(2)
            ]

            def outproj(chn, i0):
                i = i0
                for nbl in range(4):
                    nb = chn * 4 + nbl
                    for oc in range(DIM // 512):
                        yp = ypsum.tile([P, 512], F32, tag="yp")
                        for hp in range(E // P):
                            nc.tensor.matmul(
                                yp,
                                lhsT=otn2[:, hp, nb * P : (nb + 1) * P],
                                rhs=woT[:, hp, oc * 512 : (oc + 1) * 512],
                                start=(hp == 0),
                                stop=(hp == E // P - 1),
                            )
                        ys = ypool.tile([P, 512], BF16, tag="ys")
                        if i % 2:
                            nc.scalar.copy(ys, yp)
                        else:
                            nc.vector.tensor_copy(ys, yp)
                        ring = nc.sync if i % 2 else nc.scalar
                        ring.dma_start(
                            out=y_d[
                                nb * P : (nb + 1) * P, oc * 512 : (oc + 1) * 512
                            ],
                            in_=ys,
                        )
                        i += 1

            outproj(0, 0)
            for mt in range(MT):
                emit_pv(ot_ps, 1, 1, mt)
            evict_ot(ot_ps, 1, 1, [nc.scalar, nc.vector])
            normalize(rbpsB, 1, 1)
            outproj(1, 8)

    nc.compile()
    return nc


_NC_CACHE = []


def _get_nc():
    if not _NC_CACHE:
        _NC_CACHE.append(build_program())
    return _NC_CACHE[0]


def make_in_maps(x, context, mask, Wq, Wkv, b_kv, Wo):
    bf = ml_dtypes.bfloat16
    x = np.asarray(x, dtype=np.float32)
    context = np.asarray(context, dtype=np.float32)
    mask = np.asarray(mask)
    Wq = np.asarray(Wq, dtype=np.float32)
    Wkv = np.asarray(Wkv, dtype=np.float32)
    b_kv = np.asarray(b_kv, dtype=np.float32)
    Wo = np.asarray(Wo, dtype=np.float32)

    def tile_w(w, dt=None):
        # [DIM, E] contraction-major -> [128, CT, E] partition-tiled
        return np.ascontiguousarray(
            w.reshape(-1, 128, w.shape[1]).transpose(1, 0, 2)
        ).astype(dt or bf)

    in_maps = []
    for b in range(B):
        xtb = np.ascontiguousarray(x[b].T).astype(bf)
        ctb = context[b].T  # [DIM, M]
        # [4, 128, CT, 512]: m-chunk-major, partition-tiled, contiguous
        ctr = np.ascontiguousarray(
            ctb.reshape(8, 128, 4, 512).transpose(2, 1, 0, 3)
        ).astype(bf)
        # [4, 128, 4, N]: mask m-tiles grouped in chunks of 4
        mtb = mask[b].T  # [M, N]
        mtr = np.ascontiguousarray(
            mtb.reshape(4, 4, 128, N).transpose(0, 2, 1, 3)
        ).astype(bf)
        for g in range(NUM_HEADS // HPC):
            sl = slice(E * g, E * (g + 1))
            wog = Wo[:, sl].T  # [E, DIM]
            wor = np.ascontiguousarray(
                wog.reshape(2, 128, DIM).transpose(1, 0, 2)
            ).astype(bf)
            in_maps.append(
                {
                    "xT": xtb,
                    "ctxr": ctr,
                    "maskr": mtr,
                    "wqT": tile_w(Wq[sl].T),
                    "wkT": tile_w(Wkv[sl].T),
                    "wvT": tile_w(Wkv[DIM + E * g : DIM + E * (g + 1)].T),
                    "woT": wor,
                    "bk": np.ascontiguousarray(b_kv[sl]),
                }
            )
    return in_maps


def combine_outputs(ys, b_kv, Wo):
    """ys: list of 8 per-core partial outputs [N, DIM], core order (b, g)."""
    b_v = np.asarray(b_kv, dtype=np.float32)[DIM:]
    ybias = np.asarray(Wo, dtype=np.float32) @ b_v  # [DIM]
    out = np.empty((B, N, DIM), dtype=np.float32)
    G = NUM_HEADS // HPC
    for b in range(B):
        acc = np.asarray(ys[G * b], dtype=np.float32).copy()  # bf16 -> f32
        for g in range(1, G):
            acc += ys[G * b + g]
        out[b] = acc + ybias[None, :]
    return out


def kernel(x, context, mask, Wq, Wkv, b_kv, Wo):
    nc = _get_nc()
    in_maps = make_in_maps(x, context, mask, Wq, Wkv, b_kv, Wo)
    res = run_bass_kernel_spmd(nc, in_maps, core_ids=list(range(8)))
    ys = [m["y"] for m in res.results]
    return combine_outputs(ys, b_kv, Wo)
